# revision 24
# baseline (speedup 1.0000x reference)
# Trainium2 Bass kernel for Mixtral block-sparse MLP with HQQ 4-bit (int32-stored)
# group-quantized weights.
#
#   gate = silu(x @ dequant(w1).T); up = x @ dequant(w3).T
#   out  = (gate * up) @ dequant(w2).T
#
# Sharding: tensor-parallel over 8 cores on the intermediate dim I=14336
# (1792 rows of w1/w3 + 1792 cols of w2 per core).  Each core computes a
# full-shape [T, H] partial of the down-projection; the host sums the 8
# partials (cheap in numpy) instead of an on-device AllReduce.
#
# The weights are dequantized AND pre-transposed on the host (numpy), then
# recompressed for the device:
#   - w1/w3: per-output-column int8 requantization.  The column scale moves
#     OUTSIDE the matmul (applied to the f32 PSUM output during the silu
#     evacuation, via a host-pre-broadcast [128, col] scale tile), so the
#     device only needs an exact int8 -> fp16 convert (plain tensor_copy,
#     split across DVE and ACT) before the fp16 matmul.  Halves w13 DMA
#     vs fp16 with no partition-broadcast ops.
#   - w2: fp8e4 (e4m3), scaled x32 into fp8 range; the act input is scaled
#     1/16 into fp8 range and the 0.5 unscale folds into the output evac.
#     Phase 2 runs DoubleRow fp8 matmuls: K=256 per instruction = 2x fp16
#     throughput (measured).  Stage-2 fp8 costs ~5e-3 rel err (vs 2.7e-4
#     all-fp16), well inside the 2e-2 gate.
# Device pipeline per core:
#   DMA w13 int8 batch -> DVE/ACT convert fp16 -> PE matmul1 (fp16, psum)
#   -> DVE col-scale + silu*up (fp8-range) -> PE transpose -> actT (fp8)
#   -> PE matmul2 (DoubleRow fp8) -> ACT evac (x0.5) -> DMA out (fp16).
# No device-side HQQ dequant, no zero-point corrections, no collectives;
# the first matmul is gated only on a 2-k-tile weight batch + 4-k-tile xT
# block, and weight DMAs stream chunk-contiguous (2-4KB per-partition
# packets, ~290 GB/s vs 200 GB/s for naive tile DMAs).

import sys
from contextlib import ExitStack

import numpy as np

sys.path.insert(0, "/opt/trn_rl_repo")

import concourse.bacc as bacc
import concourse.mybir as mybir
import concourse.tile as tile

P = 128
GS = 64  # HQQ quant group size (along each weight's input dim)
F32 = mybir.dt.float32
AF = mybir.ActivationFunctionType
ALU = mybir.AluOpType
FP16 = mybir.dt.float16
FP8 = mybir.dt.float8e4
I8 = mybir.dt.int8
DR = mybir.MatmulPerfMode.DoubleRow

T, H, I, NCORES = 256, 4096, 14336, 8
IS = I // NCORES          # 1792 intermediate rows/cols per core
KT = H // P               # 32 k-tiles for matmul1
IT = IS // P              # 14 i-tiles for matmul2
TT = T // P               # 2 token tiles
CW = 512                  # matmul free-dim chunk width (1 psum bank of f32)
NC1 = 2 * IS // CW        # 7 chunks of interleaved [w1|w3] columns
NH2 = H // CW             # 8 output column chunks


def build_mlp_kernel(tc, outs, ins, cfg):
    nc = tc.nc
    w13 = ins["w13"]        # [P, NC1, KT, CW] int8 (col-requantized, interleaved)
    cs = ins["cs"]          # [P, NC1, CW] f32 col scales (pre-broadcast)
    w2 = ins["w2"]          # [P, NH2, IT//2, 2, CW] fp8e4 (DR-paired)
    xT = ins["xT"]          # [P, KT, T] fp16
    out_ext = outs["out"]   # [T, H] fp16 partial (host sums cores)

    ctx = ExitStack()
    with ctx:
        const = ctx.enter_context(tc.tile_pool(name="const", bufs=1))
        pst = ctx.enter_context(tc.tile_pool(name="pst", bufs=2, space="PSUM"))
        pout = ctx.enter_context(tc.tile_pool(name="pout", bufs=3, space="PSUM"))
        wtp = ctx.enter_context(tc.tile_pool(name="wt", bufs=int(cfg.get("WBUFS", 16))))
        wcp = ctx.enter_context(tc.tile_pool(name="wc", bufs=int(cfg.get("CBUFS", 8))))
        miscp = ctx.enter_context(tc.tile_pool(name="misc", bufs=4))
        obp = ctx.enter_context(tc.tile_pool(name="ob", bufs=8))

        KBLK = int(cfg.get("KBLK", 4))
        XBLK = KBLK
        NXB = KT // XBLK

        ident = const.tile([P, P], FP16, name="ident")
        nc.scalar.dma_start(ident, ins["ident"])
        # xT block 0 first: it gates the first matmul.  Blocks 1.. are
        # DMA'd after chunk 0's first weight batch (see phase 1 loop).
        xTb = [
            const.tile([P, XBLK, T], FP16, tag=f"xT{b}", name="xt")
            for b in range(NXB)
        ]
        nc.sync.dma_start(xTb[0], xT[:, 0:XBLK])

        cs_sb = const.tile([P, NC1, CW], F32, name="cs_sb")
        _pend_dma = [
            lambda b=b: nc.scalar.dma_start(xTb[b], xT[:, b * XBLK:(b + 1) * XBLK])
            for b in range(1, NXB)
        ] + [lambda: nc.scalar.dma_start(cs_sb, cs)]

        actT_sb = const.tile([P, IT, T], FP8, name="actT_sb")

        pend = []  # deferred tail work (SW pipeline: keeps PE stream dense)

        # ================= phase 1: gate/up + silu*up ====================
        for c in range(NC1):
            pot = pout.tile([P, TT, CW], F32, tag="po", name="pot")
            po = [pot[:, t] for t in range(TT)]
            kb = [2, 2] + [KBLK] * ((KT - 4) // KBLK) if c == 0 else [KBLK] * (KT // KBLK)
            wbs = []   # (k_start, fp16 tile) per batch
            k0 = 0
            for b, nk in enumerate(kb):
                wb = wtp.tile([P, KBLK, CW], I8, tag="wt", name="wb")
                # early chunks: alternate queues so the idle Scalar DMA queue
                # doubles warmup bandwidth
                eng = nc.scalar if (c < 2 and b % 2 == 1) else nc.sync
                eng.dma_start(wb[:, :nk], w13[:, c, k0:k0 + nk, :])
                if _pend_dma:
                    _pend_dma.pop(0)()
                # int8 -> fp16 convert, split across DVE and ACT
                wc = wcp.tile([P, KBLK, CW], FP16, tag="wc", name="wc")
                nh = nk if (c == 0 and b < 2) else nk // 2
                nc.vector.tensor_copy(out=wc[:, :nh], in_=wb[:, :nh])
                if nh < nk:
                    nc.scalar.activation(wc[:, nh:nk], wb[:, nh:nk], AF.Copy)
                wbs.append((k0, wc))
                k0 += nk
            bi = 0
            for k in range(KT):
                if bi + 1 < len(wbs) and k >= wbs[bi + 1][0]:
                    bi += 1
                for t in range(TT):
                    nc.tensor.matmul(
                        po[t],
                        lhsT=xTb[k // XBLK][:, k % XBLK, t * P:(t + 1) * P],
                        rhs=wbs[bi][1][:, k - wbs[bi][0], :],
                        start=(k == 0),
                        stop=(k == KT - 1),
                    )

            def act_chain(po=po, c=c):
                IC = CW // 2  # 256 gate + 256 up columns
                for t in range(TT):
                    gup = miscp.tile([P, CW], F32, tag="gup", name="gup")
                    nc.vector.tensor_tensor(gup, po[t], cs_sb[:, c], ALU.mult)
                    sig = miscp.tile([P, IC], F32, tag="sig", name="sig")
                    nc.scalar.activation(sig, gup[:, :IC], AF.Sigmoid)
                    silu = miscp.tile([P, IC], F32, tag="silu", name="silu")
                    nc.vector.tensor_tensor(silu, gup[:, :IC], sig, ALU.mult)
                    acth = miscp.tile([P, IC], FP16, tag="acth", name="acth")
                    nc.vector.scalar_tensor_tensor(
                        acth, gup[:, IC:], 1.0 / 16.0, silu, ALU.mult, ALU.mult
                    )
                    ps = pst.tile([P, IC], FP16, tag="pst", name="ps")
                    for h in range(2):
                        nc.tensor.transpose(
                            ps[:, h * P:(h + 1) * P],
                            acth[:, h * P:(h + 1) * P],
                            ident,
                        )
                    nc.any.tensor_copy(
                        out=actT_sb[:, 2 * c:2 * c + 2, t * P:(t + 1) * P],
                        in_=ps.rearrange("p (a b) -> p a b", b=P),
                    )

            pend.append(act_chain)
            if len(pend) >= 2:
                pend.pop(0)()
        while pend:
            pend.pop(0)()

        # ================= phase 2: down-projection partial ==============
        NP2 = IT // 2  # 7 DoubleRow k-pairs

        def _dma_w2(hc):
            wb = wtp.tile([P, NP2, 2, CW], FP8, tag="wt", name="wb2")
            nc.sync.dma_start(wb, w2[:, hc])
            return wb

        wbs2 = [_dma_w2(0)]
        for hc in range(NH2):
            if hc + 1 < NH2:
                wbs2.append(_dma_w2(hc + 1))
            pot2 = pout.tile([P, TT, CW], F32, tag="po", name="pot2")
            po2 = [pot2[:, t] for t in range(TT)]
            wb = wbs2[hc]
            for b in range(NP2):
                for t in range(TT):
                    nc.tensor.matmul(
                        po2[t],
                        lhsT=actT_sb[:, 2 * b:2 * b + 2, t * P:(t + 1) * P],
                        rhs=wb[:, b],
                        start=(b == 0),
                        stop=(b == NP2 - 1),
                        perf_mode=DR,
                    )

            def tail2(po2=po2, hc=hc):
                for t in range(TT):
                    ob = obp.tile([P, CW], FP16, tag="ob", name="ob")
                    nc.scalar.activation(ob, po2[t], AF.Copy, scale=0.5)
                    nc.scalar.dma_start(
                        out_ext[t * P:(t + 1) * P, hc * CW:(hc + 1) * CW], ob
                    )

            pend.append(tail2)
            if len(pend) >= 2:
                pend.pop(0)()
        while pend:
            pend.pop(0)()


# ---------------------------------------------------------------------------
# host side
# ---------------------------------------------------------------------------

FULL_CFG = dict(WBUFS=16, KBLK=4)


def build_nc(cfg):
    nc = bacc.Bacc(
        "TRN2",
        target_bir_lowering=False,
        debug=False,
        enable_asserts=False,
        num_devices=NCORES,
    )
    ins = {
        "xT": nc.dram_tensor("xT", [P, KT, T], FP16, kind="ExternalInput").ap(),
        "ident": nc.dram_tensor("ident", [P, P], FP16, kind="ExternalInput").ap(),
        "w13": nc.dram_tensor("w13", [P, NC1, KT, CW], I8, kind="ExternalInput").ap(),
        "cs": nc.dram_tensor("cs", [P, NC1, CW], F32, kind="ExternalInput").ap(),
        "w2": nc.dram_tensor("w2", [P, NH2, IT // 2, 2, CW], FP8, kind="ExternalInput").ap(),
    }
    outs = {"out": nc.dram_tensor("out", [T, H], FP16, kind="ExternalOutput").ap()}
    with tile.TileContext(nc) as tc:
        build_mlp_kernel(tc, outs, ins, cfg)
    nc.compile()
    return nc


def _dequant(wq, scale, zero):
    out_dim, in_dim = wq.shape
    g = in_dim // GS
    w = (wq.astype(np.float32).reshape(out_dim, g, GS) - zero[:, :, None]) \
        * scale[:, :, None]
    return w.reshape(out_dim, in_dim)


def _stripe(a, nt):
    # [(k p), n] -> [p, k, n] so each SBUF partition's data is contiguous
    return np.ascontiguousarray(
        a.reshape(nt, P, a.shape[1]).transpose(1, 0, 2)
    )


def make_in_maps(inputs):
    x = np.asarray(inputs["x"], dtype=np.float32)
    xT = _stripe(np.ascontiguousarray(x.T).astype(np.float16), KT)
    ident_np = np.eye(P, dtype=np.float16)

    w1 = _dequant(inputs["w1_q"], inputs["w1_scale"], inputs["w1_zero"])
    w3 = _dequant(inputs["w3_q"], inputs["w3_scale"], inputs["w3_zero"])
    w2 = _dequant(inputs["w2_q"], inputs["w2_scale"], inputs["w2_zero"])

    IC = CW // 2
    in_maps = []
    for c in range(NCORES):
        sl = slice(c * IS, (c + 1) * IS)
        w1T = w1[sl].T   # [H, IS] f32
        w3T = w3[sl].T   # [H, IS] f32
        # interleave [w1 | w3] in IC-column blocks so each CW chunk is
        # [gate cols | up cols]
        w13T = np.empty((H, 2 * IS), dtype=np.float32)
        w13Tv = w13T.reshape(H, NC1, 2, IC)
        w13Tv[:, :, 0, :] = w1T.reshape(H, NC1, IC)
        w13Tv[:, :, 1, :] = w3T.reshape(H, NC1, IC)
        # per-column int8 re-quantization; scale applied to the psum output
        colmax = np.abs(w13T).max(axis=0)
        colmax[colmax == 0] = 1.0
        w13q = np.round(w13T / colmax * 127.0).astype(np.int8)
        csB = np.ascontiguousarray(
            np.broadcast_to(
                (colmax / 127.0).astype(np.float32).reshape(NC1, CW), (P, NC1, CW)
            )
        )
        w2T = np.ascontiguousarray(w2[:, sl].T) * 32.0  # [IS, H] f32, fp8-range scaled
        # [p, k, c*CW] -> [p, c, k, CW] chunk-contiguous per partition
        w13_s = np.ascontiguousarray(
            _stripe(w13q, KT).reshape(P, KT, NC1, CW).transpose(0, 2, 1, 3)
        )
        import ml_dtypes
        w2_s = np.ascontiguousarray(
            _stripe(w2T.astype(np.float32), IT)
            .reshape(P, IT, NH2, CW)
            .transpose(0, 2, 1, 3)                  # [P, hc, ik, CW]
            .reshape(P, NH2, IT // 2, 2, CW)        # ik -> (pair, j)
        ).astype(ml_dtypes.float8_e4m3fn)
        in_maps.append(
            {
                "xT": xT,
                "ident": ident_np,
                "w13": w13_s,
                "cs": csB,
                "w2": w2_s,
            }
        )
    return in_maps


_CACHE = {}


def run_on_hw(inputs, cfg=None, trace=False, trace_kwargs=None):
    from concourse.bass_utils import run_bass_kernel_spmd

    cfg = dict(FULL_CFG if cfg is None else cfg)
    key = tuple(sorted(cfg.items()))
    if key not in _CACHE:
        _CACHE[key] = build_nc(cfg)
    nc = _CACHE[key]
    in_maps = make_in_maps(inputs)
    res = run_bass_kernel_spmd(
        nc,
        in_maps,
        list(range(NCORES)),
        trace=trace,
        **(trace_kwargs or {}),
    )
    return res


def gather_out(res):
    return np.sum(
        [np.asarray(res.results[c]["out"], dtype=np.float32) for c in range(NCORES)],
        axis=0,
    )


def kernel(**inputs) -> np.ndarray:
    res = run_on_hw(inputs)
    return gather_out(res)


# revision 25
# speedup vs baseline: 1.0626x; 1.0626x over previous
# Trainium2 Bass kernel for Mixtral block-sparse MLP with HQQ 4-bit (int32-stored)
# group-quantized weights.
#
#   gate = silu(x @ dequant(w1).T); up = x @ dequant(w3).T
#   out  = (gate * up) @ dequant(w2).T
#
# Sharding: tensor-parallel over 8 cores on the intermediate dim I=14336
# (1792 rows of w1/w3 + 1792 cols of w2 per core).  Each core computes a
# full-shape [T, H] partial of the down-projection; the host sums the 8
# partials (cheap in numpy) instead of an on-device AllReduce.
#
# The weights are dequantized AND pre-transposed on the host (numpy), then
# recompressed for the device:
#   - w1/w3: per-output-column int8 requantization.  The column scale moves
#     OUTSIDE the matmul (applied to the f32 PSUM output during the silu
#     evacuation, via a host-pre-broadcast [128, col] scale tile), so the
#     device only needs an exact int8 -> fp16 convert (plain tensor_copy,
#     split across DVE and ACT) before the fp16 matmul.  Halves w13 DMA
#     vs fp16 with no partition-broadcast ops.
#   - w2: fp8e4 (e4m3), scaled x32 into fp8 range; the act input is scaled
#     1/16 into fp8 range and the 0.5 unscale folds into the output evac.
#     Phase 2 runs DoubleRow fp8 matmuls: K=256 per instruction = 2x fp16
#     throughput (measured).  Stage-2 fp8 costs ~5e-3 rel err (vs 2.7e-4
#     all-fp16), well inside the 2e-2 gate.
# Device pipeline per core:
#   DMA w13 int8 batch -> DVE/ACT convert fp16 -> PE matmul1 (fp16, psum)
#   -> DVE col-scale + silu*up (fp8-range) -> PE transpose -> actT (fp8)
#   -> PE matmul2 (DoubleRow fp8) -> ACT evac (x0.5) -> DMA out (fp16).
# No device-side HQQ dequant, no zero-point corrections, no collectives;
# the first matmul is gated only on a 2-k-tile weight batch + 4-k-tile xT
# block, and weight DMAs stream chunk-contiguous (2-4KB per-partition
# packets, ~290 GB/s vs 200 GB/s for naive tile DMAs).

import sys
from contextlib import ExitStack

import numpy as np

sys.path.insert(0, "/opt/trn_rl_repo")

import concourse.bacc as bacc
import concourse.mybir as mybir
import concourse.tile as tile

P = 128
GS = 64  # HQQ quant group size (along each weight's input dim)
F32 = mybir.dt.float32
AF = mybir.ActivationFunctionType
ALU = mybir.AluOpType
FP16 = mybir.dt.float16
FP8 = mybir.dt.float8e4
I8 = mybir.dt.int8
DR = mybir.MatmulPerfMode.DoubleRow

T, H, I, NCORES = 256, 4096, 14336, 8
IS = I // NCORES          # 1792 intermediate rows/cols per core
KT = H // P               # 32 k-tiles for matmul1
IT = IS // P              # 14 i-tiles for matmul2
TT = T // P               # 2 token tiles
CW = 512                  # matmul free-dim chunk width (1 psum bank of f32)
NC1 = 2 * IS // CW        # 7 chunks of interleaved [w1|w3] columns
NH2 = H // CW             # 8 output column chunks


def build_mlp_kernel(tc, outs, ins, cfg):
    nc = tc.nc
    w13 = ins["w13"]        # [P, NC1, KT, CW] int8 (col-requantized, interleaved)
    cs = ins["cs"]          # [P, NC1, CW] f32 col scales (pre-broadcast)
    w2 = ins["w2"]          # [P, NH2, IT//2, 2, CW] fp8e4 (DR-paired)
    xT = ins["xT"]          # [P, KT, T] fp16
    out_ext = outs["out"]   # [T, H] fp16 partial (host sums cores)

    ctx = ExitStack()
    with ctx:
        const = ctx.enter_context(tc.tile_pool(name="const", bufs=1))
        pst = ctx.enter_context(tc.tile_pool(name="pst", bufs=2, space="PSUM"))
        pout = ctx.enter_context(tc.tile_pool(name="pout", bufs=3, space="PSUM"))
        wtp = ctx.enter_context(tc.tile_pool(name="wt", bufs=int(cfg.get("WBUFS", 16))))
        wcp = ctx.enter_context(tc.tile_pool(name="wc", bufs=int(cfg.get("CBUFS", 8))))
        miscp = ctx.enter_context(tc.tile_pool(name="misc", bufs=4))
        obp = ctx.enter_context(tc.tile_pool(name="ob", bufs=8))

        KBLK = int(cfg.get("KBLK", 4))
        XBLK = KBLK
        NXB = KT // XBLK

        ident = const.tile([P, P], FP16, name="ident")
        nc.scalar.dma_start(ident, ins["ident"])
        # xT block 0 first: it gates the first matmul.  Blocks 1.. are
        # DMA'd after chunk 0's first weight batch (see phase 1 loop).
        xTb = [
            const.tile([P, XBLK, T], FP16, tag=f"xT{b}", name="xt")
            for b in range(NXB)
        ]
        nc.sync.dma_start(xTb[0], xT[:, 0:XBLK])

        cs_sb = const.tile([P, NC1, CW], F32, name="cs_sb")
        _pend_dma = [
            lambda b=b: nc.scalar.dma_start(xTb[b], xT[:, b * XBLK:(b + 1) * XBLK])
            for b in range(1, NXB)
        ] + [lambda: nc.scalar.dma_start(cs_sb, cs)]

        actT_sb = const.tile([P, IT, T], FP8, name="actT_sb")

        pend = []  # deferred tail work (SW pipeline: keeps PE stream dense)

        # ================= phase 1: gate/up + silu*up ====================
        for c in range(NC1):
            pot = pout.tile([P, TT, CW], F32, tag="po", name="pot")
            po = [pot[:, t] for t in range(TT)]
            kb = [2, 2] + [KBLK] * ((KT - 4) // KBLK) if c == 0 else [KBLK] * (KT // KBLK)
            wbs = []   # (k_start, fp16 tile) per batch
            k0 = 0
            for b, nk in enumerate(kb):
                wb = wtp.tile([P, KBLK, CW], I8, tag="wt", name="wb")
                nc.sync.dma_start(wb[:, :nk], w13[:, c, k0:k0 + nk, :])
                if _pend_dma:
                    _pend_dma.pop(0)()
                # int8 -> fp16 convert, split across DVE and ACT
                wc = wcp.tile([P, KBLK, CW], FP16, tag="wc", name="wc")
                nh = nk if (c == 0 and b < 2) else nk // 2
                nc.vector.tensor_copy(out=wc[:, :nh], in_=wb[:, :nh])
                if nh < nk:
                    nc.scalar.activation(wc[:, nh:nk], wb[:, nh:nk], AF.Copy)
                wbs.append((k0, wc))
                k0 += nk
            bi = 0
            for k in range(KT):
                if bi + 1 < len(wbs) and k >= wbs[bi + 1][0]:
                    bi += 1
                for t in range(TT):
                    nc.tensor.matmul(
                        po[t],
                        lhsT=xTb[k // XBLK][:, k % XBLK, t * P:(t + 1) * P],
                        rhs=wbs[bi][1][:, k - wbs[bi][0], :],
                        start=(k == 0),
                        stop=(k == KT - 1),
                    )

            def act_chain(po=po, c=c):
                IC = CW // 2  # 256 gate + 256 up columns
                for t in range(TT):
                    gup = miscp.tile([P, CW], F32, tag="gup", name="gup")
                    nc.vector.tensor_tensor(gup, po[t], cs_sb[:, c], ALU.mult)
                    sig = miscp.tile([P, IC], F32, tag="sig", name="sig")
                    nc.scalar.activation(sig, gup[:, :IC], AF.Sigmoid)
                    silu = miscp.tile([P, IC], F32, tag="silu", name="silu")
                    nc.vector.tensor_tensor(silu, gup[:, :IC], sig, ALU.mult)
                    acth = miscp.tile([P, IC], FP16, tag="acth", name="acth")
                    nc.vector.scalar_tensor_tensor(
                        acth, gup[:, IC:], 1.0 / 16.0, silu, ALU.mult, ALU.mult
                    )
                    ps = pst.tile([P, IC], FP16, tag="pst", name="ps")
                    for h in range(2):
                        nc.tensor.transpose(
                            ps[:, h * P:(h + 1) * P],
                            acth[:, h * P:(h + 1) * P],
                            ident,
                        )
                    nc.any.tensor_copy(
                        out=actT_sb[:, 2 * c:2 * c + 2, t * P:(t + 1) * P],
                        in_=ps.rearrange("p (a b) -> p a b", b=P),
                    )

            pend.append(act_chain)
            if len(pend) >= 2:
                pend.pop(0)()
        while pend:
            pend.pop(0)()

        # ================= phase 2: down-projection partial ==============
        NP2 = IT // 2  # 7 DoubleRow k-pairs

        def _dma_w2(hc):
            wb = wtp.tile([P, NP2, 2, CW], FP8, tag="wt", name="wb2")
            nc.sync.dma_start(wb, w2[:, hc])
            return wb

        wbs2 = [_dma_w2(0)]
        for hc in range(NH2):
            if hc + 1 < NH2:
                wbs2.append(_dma_w2(hc + 1))
            pot2 = pout.tile([P, TT, CW], F32, tag="po", name="pot2")
            po2 = [pot2[:, t] for t in range(TT)]
            wb = wbs2[hc]
            for b in range(NP2):
                for t in range(TT):
                    nc.tensor.matmul(
                        po2[t],
                        lhsT=actT_sb[:, 2 * b:2 * b + 2, t * P:(t + 1) * P],
                        rhs=wb[:, b],
                        start=(b == 0),
                        stop=(b == NP2 - 1),
                        perf_mode=DR,
                    )

            def tail2(po2=po2, hc=hc):
                for t in range(TT):
                    ob = obp.tile([P, CW], FP16, tag="ob", name="ob")
                    nc.scalar.activation(ob, po2[t], AF.Copy, scale=0.5)
                    nc.scalar.dma_start(
                        out_ext[t * P:(t + 1) * P, hc * CW:(hc + 1) * CW], ob
                    )

            pend.append(tail2)
            if len(pend) >= 2:
                pend.pop(0)()
        while pend:
            pend.pop(0)()


# ---------------------------------------------------------------------------
# host side
# ---------------------------------------------------------------------------

FULL_CFG = dict(WBUFS=16, KBLK=4)


def build_nc(cfg):
    nc = bacc.Bacc(
        "TRN2",
        target_bir_lowering=False,
        debug=False,
        enable_asserts=False,
        num_devices=NCORES,
    )
    ins = {
        "xT": nc.dram_tensor("xT", [P, KT, T], FP16, kind="ExternalInput").ap(),
        "ident": nc.dram_tensor("ident", [P, P], FP16, kind="ExternalInput").ap(),
        "w13": nc.dram_tensor("w13", [P, NC1, KT, CW], I8, kind="ExternalInput").ap(),
        "cs": nc.dram_tensor("cs", [P, NC1, CW], F32, kind="ExternalInput").ap(),
        "w2": nc.dram_tensor("w2", [P, NH2, IT // 2, 2, CW], FP8, kind="ExternalInput").ap(),
    }
    outs = {"out": nc.dram_tensor("out", [T, H], FP16, kind="ExternalOutput").ap()}
    with tile.TileContext(nc) as tc:
        build_mlp_kernel(tc, outs, ins, cfg)
    nc.compile()
    return nc


def _dequant(wq, scale, zero):
    out_dim, in_dim = wq.shape
    g = in_dim // GS
    w = (wq.astype(np.float32).reshape(out_dim, g, GS) - zero[:, :, None]) \
        * scale[:, :, None]
    return w.reshape(out_dim, in_dim)


def _stripe(a, nt):
    # [(k p), n] -> [p, k, n] so each SBUF partition's data is contiguous
    return np.ascontiguousarray(
        a.reshape(nt, P, a.shape[1]).transpose(1, 0, 2)
    )


def make_in_maps(inputs):
    x = np.asarray(inputs["x"], dtype=np.float32)
    xT = _stripe(np.ascontiguousarray(x.T).astype(np.float16), KT)
    ident_np = np.eye(P, dtype=np.float16)

    w1 = _dequant(inputs["w1_q"], inputs["w1_scale"], inputs["w1_zero"])
    w3 = _dequant(inputs["w3_q"], inputs["w3_scale"], inputs["w3_zero"])
    w2 = _dequant(inputs["w2_q"], inputs["w2_scale"], inputs["w2_zero"])

    IC = CW // 2
    in_maps = []
    for c in range(NCORES):
        sl = slice(c * IS, (c + 1) * IS)
        w1T = w1[sl].T   # [H, IS] f32
        w3T = w3[sl].T   # [H, IS] f32
        # interleave [w1 | w3] in IC-column blocks so each CW chunk is
        # [gate cols | up cols]
        w13T = np.empty((H, 2 * IS), dtype=np.float32)
        w13Tv = w13T.reshape(H, NC1, 2, IC)
        w13Tv[:, :, 0, :] = w1T.reshape(H, NC1, IC)
        w13Tv[:, :, 1, :] = w3T.reshape(H, NC1, IC)
        # per-column int8 re-quantization; scale applied to the psum output
        colmax = np.abs(w13T).max(axis=0)
        colmax[colmax == 0] = 1.0
        w13q = np.round(w13T / colmax * 127.0).astype(np.int8)
        csB = np.ascontiguousarray(
            np.broadcast_to(
                (colmax / 127.0).astype(np.float32).reshape(NC1, CW), (P, NC1, CW)
            )
        )
        w2T = np.ascontiguousarray(w2[:, sl].T) * 32.0  # [IS, H] f32, fp8-range scaled
        # [p, k, c*CW] -> [p, c, k, CW] chunk-contiguous per partition
        w13_s = np.ascontiguousarray(
            _stripe(w13q, KT).reshape(P, KT, NC1, CW).transpose(0, 2, 1, 3)
        )
        import ml_dtypes
        w2_s = np.ascontiguousarray(
            _stripe(w2T.astype(np.float32), IT)
            .reshape(P, IT, NH2, CW)
            .transpose(0, 2, 1, 3)                  # [P, hc, ik, CW]
            .reshape(P, NH2, IT // 2, 2, CW)        # ik -> (pair, j)
        ).astype(ml_dtypes.float8_e4m3fn)
        in_maps.append(
            {
                "xT": xT,
                "ident": ident_np,
                "w13": w13_s,
                "cs": csB,
                "w2": w2_s,
            }
        )
    return in_maps


_CACHE = {}


def run_on_hw(inputs, cfg=None, trace=False, trace_kwargs=None):
    from concourse.bass_utils import run_bass_kernel_spmd

    cfg = dict(FULL_CFG if cfg is None else cfg)
    key = tuple(sorted(cfg.items()))
    if key not in _CACHE:
        _CACHE[key] = build_nc(cfg)
    nc = _CACHE[key]
    in_maps = make_in_maps(inputs)
    res = run_bass_kernel_spmd(
        nc,
        in_maps,
        list(range(NCORES)),
        trace=trace,
        **(trace_kwargs or {}),
    )
    return res


def gather_out(res):
    return np.sum(
        [np.asarray(res.results[c]["out"], dtype=np.float32) for c in range(NCORES)],
        axis=0,
    )


def kernel(**inputs) -> np.ndarray:
    res = run_on_hw(inputs)
    return gather_out(res)


# revision 26
# speedup vs baseline: 1.0640x; 1.0013x over previous
# Trainium2 Bass kernel for Mixtral block-sparse MLP with HQQ 4-bit (int32-stored)
# group-quantized weights.
#
#   gate = silu(x @ dequant(w1).T); up = x @ dequant(w3).T
#   out  = (gate * up) @ dequant(w2).T
#
# Sharding: tensor-parallel over 8 cores on the intermediate dim I=14336
# (1792 rows of w1/w3 + 1792 cols of w2 per core).  Each core computes a
# full-shape [T, H] partial of the down-projection; the host sums the 8
# partials (cheap in numpy) instead of an on-device AllReduce.
#
# The weights are dequantized AND pre-transposed on the host (numpy), then
# recompressed for the device:
#   - w1/w3: per-output-column int8 requantization.  The column scale moves
#     OUTSIDE the matmul (applied to the f32 PSUM output during the silu
#     evacuation, via a host-pre-broadcast [128, col] scale tile), so the
#     device only needs an exact int8 -> fp16 convert (plain tensor_copy,
#     split across DVE and ACT) before the fp16 matmul.  Halves w13 DMA
#     vs fp16 with no partition-broadcast ops.
#   - w2: fp8e4 (e4m3), scaled x32 into fp8 range; the act input is scaled
#     1/16 into fp8 range and the 0.5 unscale folds into the output evac.
#     Phase 2 runs DoubleRow fp8 matmuls: K=256 per instruction = 2x fp16
#     throughput (measured).  Stage-2 fp8 costs ~5e-3 rel err (vs 2.7e-4
#     all-fp16), well inside the 2e-2 gate.
# Device pipeline per core:
#   DMA w13 int8 batch -> DVE/ACT convert fp16 -> PE matmul1 (fp16, psum)
#   -> DVE col-scale + silu*up (fp8-range) -> PE transpose -> actT (fp8)
#   -> PE matmul2 (DoubleRow fp8) -> ACT evac (x0.5) -> DMA out (fp16).
# No device-side HQQ dequant, no zero-point corrections, no collectives;
# the first matmul is gated only on a 2-k-tile weight batch + 4-k-tile xT
# block, and weight DMAs stream chunk-contiguous (2-4KB per-partition
# packets, ~290 GB/s vs 200 GB/s for naive tile DMAs).

import sys
from contextlib import ExitStack

import numpy as np

sys.path.insert(0, "/opt/trn_rl_repo")

import concourse.bacc as bacc
import concourse.mybir as mybir
import concourse.tile as tile

P = 128
GS = 64  # HQQ quant group size (along each weight's input dim)
F32 = mybir.dt.float32
AF = mybir.ActivationFunctionType
ALU = mybir.AluOpType
FP16 = mybir.dt.float16
FP8 = mybir.dt.float8e4
I8 = mybir.dt.int8
DR = mybir.MatmulPerfMode.DoubleRow

T, H, I, NCORES = 256, 4096, 14336, 8
IS = I // NCORES          # 1792 intermediate rows/cols per core
KT = H // P               # 32 k-tiles for matmul1
IT = IS // P              # 14 i-tiles for matmul2
TT = T // P               # 2 token tiles
CW = 512                  # matmul free-dim chunk width (1 psum bank of f32)
NC1 = 2 * IS // CW        # 7 chunks of interleaved [w1|w3] columns
NH2 = H // CW             # 8 output column chunks


def build_mlp_kernel(tc, outs, ins, cfg):
    nc = tc.nc
    w13 = ins["w13"]        # [P, NC1, KT, CW] int8 (col-requantized, interleaved)
    cs = ins["cs"]          # [P, NC1, CW] f32 col scales (pre-broadcast)
    w2 = ins["w2"]          # [P, NH2, IT//2, 2, CW] fp8e4 (DR-paired)
    xT = ins["xT"]          # [P, KT, T] fp16
    out_ext = outs["out"]   # [T, H] fp16 partial (host sums cores)

    ctx = ExitStack()
    with ctx:
        const = ctx.enter_context(tc.tile_pool(name="const", bufs=1))
        pst = ctx.enter_context(tc.tile_pool(name="pst", bufs=2, space="PSUM"))
        pout = ctx.enter_context(tc.tile_pool(name="pout", bufs=3, space="PSUM"))
        wtp = ctx.enter_context(tc.tile_pool(name="wt", bufs=int(cfg.get("WBUFS", 16))))
        wcp = ctx.enter_context(tc.tile_pool(name="wc", bufs=int(cfg.get("CBUFS", 8))))
        miscp = ctx.enter_context(tc.tile_pool(name="misc", bufs=4))
        obp = ctx.enter_context(tc.tile_pool(name="ob", bufs=8))

        KBLK = int(cfg.get("KBLK", 4))
        XBLK = KBLK
        NXB = KT // XBLK

        ident = const.tile([P, P], FP16, name="ident")
        nc.scalar.dma_start(ident, ins["ident"])
        # xT block 0 first: it gates the first matmul.  Blocks 1.. are
        # DMA'd after chunk 0's first weight batch (see phase 1 loop).
        xTb = [
            const.tile([P, XBLK, T], FP16, tag=f"xT{b}", name="xt")
            for b in range(NXB)
        ]
        nc.sync.dma_start(xTb[0], xT[:, 0:XBLK])

        cs_sb = const.tile([P, NC1, CW], FP16, name="cs_sb")
        _pend_dma = [
            lambda b=b: nc.scalar.dma_start(xTb[b], xT[:, b * XBLK:(b + 1) * XBLK])
            for b in range(1, NXB)
        ] + [lambda: nc.scalar.dma_start(cs_sb, cs)]

        actT_sb = const.tile([P, IT, T], FP8, name="actT_sb")

        pend = []  # deferred tail work (SW pipeline: keeps PE stream dense)

        # ================= phase 1: gate/up + silu*up ====================
        for c in range(NC1):
            pot = pout.tile([P, TT, CW], F32, tag="po", name="pot")
            po = [pot[:, t] for t in range(TT)]
            kb = [2, 2] + [KBLK] * ((KT - 4) // KBLK) if c == 0 else [KBLK] * (KT // KBLK)
            wbs = []   # (k_start, fp16 tile) per batch
            k0 = 0
            for b, nk in enumerate(kb):
                wb = wtp.tile([P, KBLK, CW], I8, tag="wt", name="wb")
                nc.sync.dma_start(wb[:, :nk], w13[:, c, k0:k0 + nk, :])
                if _pend_dma:
                    _pend_dma.pop(0)()
                # int8 -> fp16 convert, split across DVE and ACT
                wc = wcp.tile([P, KBLK, CW], FP16, tag="wc", name="wc")
                nh = nk if (c == 0 and b < 2) else nk // 2
                nc.vector.tensor_copy(out=wc[:, :nh], in_=wb[:, :nh])
                if nh < nk:
                    nc.scalar.activation(wc[:, nh:nk], wb[:, nh:nk], AF.Copy)
                wbs.append((k0, wc))
                k0 += nk
            bi = 0
            for k in range(KT):
                if bi + 1 < len(wbs) and k >= wbs[bi + 1][0]:
                    bi += 1
                for t in range(TT):
                    nc.tensor.matmul(
                        po[t],
                        lhsT=xTb[k // XBLK][:, k % XBLK, t * P:(t + 1) * P],
                        rhs=wbs[bi][1][:, k - wbs[bi][0], :],
                        start=(k == 0),
                        stop=(k == KT - 1),
                    )

            def act_chain(po=po, c=c):
                IC = CW // 2  # 256 gate + 256 up columns
                for t in range(TT):
                    gup = miscp.tile([P, CW], F32, tag="gup", name="gup")
                    nc.vector.tensor_tensor(gup, po[t], cs_sb[:, c], ALU.mult)
                    sig = miscp.tile([P, IC], F32, tag="sig", name="sig")
                    nc.scalar.activation(sig, gup[:, :IC], AF.Sigmoid)
                    silu = miscp.tile([P, IC], F32, tag="silu", name="silu")
                    nc.vector.tensor_tensor(silu, gup[:, :IC], sig, ALU.mult)
                    acth = miscp.tile([P, IC], FP16, tag="acth", name="acth")
                    nc.vector.scalar_tensor_tensor(
                        acth, gup[:, IC:], 1.0 / 16.0, silu, ALU.mult, ALU.mult
                    )
                    ps = pst.tile([P, IC], FP16, tag="pst", name="ps")
                    for h in range(2):
                        nc.tensor.transpose(
                            ps[:, h * P:(h + 1) * P],
                            acth[:, h * P:(h + 1) * P],
                            ident,
                        )
                    nc.any.tensor_copy(
                        out=actT_sb[:, 2 * c:2 * c + 2, t * P:(t + 1) * P],
                        in_=ps.rearrange("p (a b) -> p a b", b=P),
                    )

            pend.append(act_chain)
            if len(pend) >= 2:
                pend.pop(0)()
        while pend:
            pend.pop(0)()

        # ================= phase 2: down-projection partial ==============
        NP2 = IT // 2  # 7 DoubleRow k-pairs

        def _dma_w2(hc):
            wb = wtp.tile([P, NP2, 2, CW], FP8, tag="wt", name="wb2")
            nc.sync.dma_start(wb, w2[:, hc])
            return wb

        wbs2 = [_dma_w2(0)]
        for hc in range(NH2):
            if hc + 1 < NH2:
                wbs2.append(_dma_w2(hc + 1))
            pot2 = pout.tile([P, TT, CW], F32, tag="po", name="pot2")
            po2 = [pot2[:, t] for t in range(TT)]
            wb = wbs2[hc]
            for b in range(NP2):
                for t in range(TT):
                    nc.tensor.matmul(
                        po2[t],
                        lhsT=actT_sb[:, 2 * b:2 * b + 2, t * P:(t + 1) * P],
                        rhs=wb[:, b],
                        start=(b == 0),
                        stop=(b == NP2 - 1),
                        perf_mode=DR,
                    )

            def tail2(po2=po2, hc=hc):
                for t in range(TT):
                    ob = obp.tile([P, CW], FP16, tag="ob", name="ob")
                    nc.scalar.activation(ob, po2[t], AF.Copy, scale=0.5)
                    nc.scalar.dma_start(
                        out_ext[t * P:(t + 1) * P, hc * CW:(hc + 1) * CW], ob
                    )

            pend.append(tail2)
            if len(pend) >= 2:
                pend.pop(0)()
        while pend:
            pend.pop(0)()


# ---------------------------------------------------------------------------
# host side
# ---------------------------------------------------------------------------

FULL_CFG = dict(WBUFS=16, KBLK=4)


def build_nc(cfg):
    nc = bacc.Bacc(
        "TRN2",
        target_bir_lowering=False,
        debug=False,
        enable_asserts=False,
        num_devices=NCORES,
    )
    ins = {
        "xT": nc.dram_tensor("xT", [P, KT, T], FP16, kind="ExternalInput").ap(),
        "ident": nc.dram_tensor("ident", [P, P], FP16, kind="ExternalInput").ap(),
        "w13": nc.dram_tensor("w13", [P, NC1, KT, CW], I8, kind="ExternalInput").ap(),
        "cs": nc.dram_tensor("cs", [P, NC1, CW], FP16, kind="ExternalInput").ap(),
        "w2": nc.dram_tensor("w2", [P, NH2, IT // 2, 2, CW], FP8, kind="ExternalInput").ap(),
    }
    outs = {"out": nc.dram_tensor("out", [T, H], FP16, kind="ExternalOutput").ap()}
    with tile.TileContext(nc) as tc:
        build_mlp_kernel(tc, outs, ins, cfg)
    nc.compile()
    return nc


def _dequant(wq, scale, zero):
    out_dim, in_dim = wq.shape
    g = in_dim // GS
    w = (wq.astype(np.float32).reshape(out_dim, g, GS) - zero[:, :, None]) \
        * scale[:, :, None]
    return w.reshape(out_dim, in_dim)


def _stripe(a, nt):
    # [(k p), n] -> [p, k, n] so each SBUF partition's data is contiguous
    return np.ascontiguousarray(
        a.reshape(nt, P, a.shape[1]).transpose(1, 0, 2)
    )


def make_in_maps(inputs):
    x = np.asarray(inputs["x"], dtype=np.float32)
    xT = _stripe(np.ascontiguousarray(x.T).astype(np.float16), KT)
    ident_np = np.eye(P, dtype=np.float16)

    w1 = _dequant(inputs["w1_q"], inputs["w1_scale"], inputs["w1_zero"])
    w3 = _dequant(inputs["w3_q"], inputs["w3_scale"], inputs["w3_zero"])
    w2 = _dequant(inputs["w2_q"], inputs["w2_scale"], inputs["w2_zero"])

    IC = CW // 2
    in_maps = []
    for c in range(NCORES):
        sl = slice(c * IS, (c + 1) * IS)
        w1T = w1[sl].T   # [H, IS] f32
        w3T = w3[sl].T   # [H, IS] f32
        # interleave [w1 | w3] in IC-column blocks so each CW chunk is
        # [gate cols | up cols]
        w13T = np.empty((H, 2 * IS), dtype=np.float32)
        w13Tv = w13T.reshape(H, NC1, 2, IC)
        w13Tv[:, :, 0, :] = w1T.reshape(H, NC1, IC)
        w13Tv[:, :, 1, :] = w3T.reshape(H, NC1, IC)
        # per-column int8 re-quantization; scale applied to the psum output
        colmax = np.abs(w13T).max(axis=0)
        colmax[colmax == 0] = 1.0
        w13q = np.round(w13T / colmax * 127.0).astype(np.int8)
        csB = np.ascontiguousarray(
            np.broadcast_to(
                (colmax / 127.0).astype(np.float16).reshape(NC1, CW), (P, NC1, CW)
            )
        )
        w2T = np.ascontiguousarray(w2[:, sl].T) * 32.0  # [IS, H] f32, fp8-range scaled
        # [p, k, c*CW] -> [p, c, k, CW] chunk-contiguous per partition
        w13_s = np.ascontiguousarray(
            _stripe(w13q, KT).reshape(P, KT, NC1, CW).transpose(0, 2, 1, 3)
        )
        import ml_dtypes
        w2_s = np.ascontiguousarray(
            _stripe(w2T.astype(np.float32), IT)
            .reshape(P, IT, NH2, CW)
            .transpose(0, 2, 1, 3)                  # [P, hc, ik, CW]
            .reshape(P, NH2, IT // 2, 2, CW)        # ik -> (pair, j)
        ).astype(ml_dtypes.float8_e4m3fn)
        in_maps.append(
            {
                "xT": xT,
                "ident": ident_np,
                "w13": w13_s,
                "cs": csB,
                "w2": w2_s,
            }
        )
    return in_maps


_CACHE = {}


def run_on_hw(inputs, cfg=None, trace=False, trace_kwargs=None):
    from concourse.bass_utils import run_bass_kernel_spmd

    cfg = dict(FULL_CFG if cfg is None else cfg)
    key = tuple(sorted(cfg.items()))
    if key not in _CACHE:
        _CACHE[key] = build_nc(cfg)
    nc = _CACHE[key]
    in_maps = make_in_maps(inputs)
    res = run_bass_kernel_spmd(
        nc,
        in_maps,
        list(range(NCORES)),
        trace=trace,
        **(trace_kwargs or {}),
    )
    return res


def gather_out(res):
    return np.sum(
        [np.asarray(res.results[c]["out"], dtype=np.float32) for c in range(NCORES)],
        axis=0,
    )


def kernel(**inputs) -> np.ndarray:
    res = run_on_hw(inputs)
    return gather_out(res)
